# revision 41
# baseline (speedup 1.0000x reference)
"""Trainium2 Bass kernel for nn_CombinedN2NWaveletLoss — v2 redesign.

Loss algebra: rec + 2*reg = 3*mean((out-c)^2) + (2/3)*mean((U delta U^T)^2)
with c = U qc U^T / 3, qc = 2*p0 + p3, delta = p0 - p3.  The delta term is
computed on the coarse grid via the Gram quadratic form sum(delta' * (G d G))
with G = U^T U (tridiagonal: diag 1.25, off 0.375, clamped edges); the row
factor (Gv/1.25, exact at the clamped edges thanks to dup guards) is staged
from the host, the column factor is 2 main + 2 halo PE matmuls per image.

Layout: transposed; partitions carry image COLUMNS, free dim carries rows.
The vertical (row) 2x upsample is folded into the PE stationaries as 3
coarse row-taps (T[par][dn] = sum_dy rho[par,dy,dn] * (K_dy @ U)), so the
conv consumes raw p0 directly.  Spurious conv zero-pad terms at fine rows
0/511 are removed by an extra 1-column matmul inside each accumulation
group.  Eviction: ACT Relu (psum->fp16) + one DVE min(.,1) per image at 4x.
The c field uses 2 row-taps (1/3 folded in) plus -I matmuls on clipped out,
evicted with ACT Square+accum.  Wavelet lvl1: batched shuffle DMAs + DVE
butterflies + ACT Abs + DVE min/accum.  lvl2/lvl3: entirely on PE via
(P_E +- P_O) permutation-butterfly stationaries with row-pair accumulation,
ACT Abs-fused psum evictions, DVE min/accum.
"""

import numpy as np

B_TOTAL = 32
N_CORES = 8
IMG = 4
H = W = 512
HC = 256
THRESHOLD = 50.0 / 255.0
GAMMA = 2.0
WAVELET_WEIGHT = 0.05
WIN = (0, 62, 126, 128)          # stationary col-window start per chunk t
PAIRS = ((0, 3), (1, 2))         # chunk pairs per unit half
MOVP = (0, 2, 3, 1)              # p0 moving plane per chunk t
CPLANE = (4, 6, 7, 5)            # qc moving plane per chunk t
NPLANE = 12                      # p0 x4; qc x4; d x2; d' x2
NACC = 18                        # 0..7 c, 8/16 lvl1P0 pos/neg, 9 lvl1P1, 10 lvl2, 11 lvl3, 12..15 delta
CTAPS = (((-1, 0.25), (0, 0.75)), ((0, 0.75), (1, 0.25)))

# stationary pack indices; conv+corr blocks grouped by unit pair so the
# first DMA covers unit half 0 (chunks t0, t3)
_PPOS = (0, 2, 3, 1)      # t -> position in pack


def st_conv(t, par, dni):
    return (_PPOS[t] * 8 + par * 3 + dni) * 128

def st_corr(t, e):
    return (_PPOS[t] * 8 + 6 + e) * 128

def st_c(t, par, j):
    # Tc[0][0]=Tc[1][1]=(0.25/3)U-window, Tc[0][1]=Tc[1][0]=(0.75/3)U-window
    big = 1 if (par == 0) == (j == 1) else 0
    return (32 + t * 2 + big) * 128

ST_I = 40 * 128

def st_g(ch, h):
    return (41 + ch * 2 + h) * 128

def st_l2(sd, th, neg):              # sd: 0=sum(P_E+P_O) 1=dif; neg: rp sign
    return (45 + sd * 2 + th + 4 * neg) * 128

def st_l3(sd, neg):
    return (53 + sd + 2 * neg) * 128

NSTAT = 57

_CACHE = {}


def _brev6(q):
    return int(f"{q:06b}"[::-1], 2)


def _lanes(t):
    idx = np.empty(128, dtype=np.int64)
    for q in range(128):
        jp = 64 * t + _brev6(q % 64)
        idx[q] = 2 * jp + (0 if q < 64 else 1)
    return idx


def _upsample_matrix():
    U = np.zeros((H, HC), dtype=np.float64)
    for j in range(H):
        src = (j + 0.5) / 2.0 - 0.5
        k0 = int(np.floor(src))
        frac = src - k0
        for k, wgt in ((k0, 1 - frac), (k0 + 1, frac)):
            U[j, min(max(k, 0), HC - 1)] += wgt
    return U


def _conv_colops(w):
    U = _upsample_matrix()
    wm = np.asarray(w, dtype=np.float64).reshape(3, 3)
    Nm = []
    for dy in (-1, 0, 1):
        K = np.zeros((H, H))
        for j in range(H):
            for dx in (-1, 0, 1):
                if 0 <= j + dx < H:
                    K[j, j + dx] = wm[dy + 1, dx + 1]
        Nm.append(K @ U)
    return Nm


def _perm_l2(eo):
    """P with P[src_lane, dst] = 1: E2/O2[32tv+s] <- ll1[(tv&1)*64+32*eo+s]."""
    P = np.zeros((128, 128))
    for tv in range(4):
        for s in range(32):
            P[(tv & 1) * 64 + 32 * eo + s, 32 * tv + s] = 1.0
    return P


def _perm_l2_th(eo, th):
    """th-masked: only tv with tv>>1 == th contribute."""
    P = np.zeros((128, 128))
    for tv in range(4):
        if (tv >> 1) != th:
            continue
        for s in range(32):
            P[(tv & 1) * 64 + 32 * eo + s, 32 * tv + s] = 1.0
    return P


def _perm_l3(eo):
    P = np.zeros((128, 128))
    for tv in range(4):
        for s in range(16):
            P[32 * tv + 16 * eo + s, 16 * tv + s] = 1.0
    return P


def _build_stats(w):
    Nm = _conv_colops(w)
    rho = np.zeros((2, 3, 3))
    rho[0, 0, 0] = 0.75; rho[0, 0, 1] = 0.25
    rho[0, 1, 0] = 0.25; rho[0, 1, 1] = 0.75
    rho[0, 2, 1] = 0.75; rho[0, 2, 2] = 0.25
    rho[1, 0, 0] = 0.25; rho[1, 0, 1] = 0.75
    rho[1, 1, 1] = 0.75; rho[1, 1, 2] = 0.25
    rho[1, 2, 1] = 0.25; rho[1, 2, 2] = 0.75
    T = [[sum(rho[par, dyi, dni] * Nm[dyi] for dyi in range(3))
          for dni in range(3)] for par in range(2)]
    U = _upsample_matrix()
    G125 = 1.25 * (U.T @ U)

    out = np.zeros((128, NSTAT * 128), dtype=np.float32)
    for t in range(4):
        J = _lanes(t)
        for par in range(2):
            for dni in range(3):
                k = st_conv(t, par, dni)
                out[:, k:k + 128] = T[par][dni][J, WIN[t]:WIN[t] + 128].T
        for big, r in ((0, 0.25), (1, 0.75)):
            k = (32 + t * 2 + big) * 128
            out[:, k:k + 128] = (r / 3.0) * U[J, WIN[t]:WIN[t] + 128].T
        # conv edge corrections (zero-pad spurious terms), 1-col matmuls
        for e, Ncor in ((0, Nm[0]), (1, Nm[2])):
            k = st_corr(t, e)
            out[:, k:k + 128] = -Ncor[J, WIN[t]:WIN[t] + 128].T
    out[:, ST_I:ST_I + 128] = -np.eye(128)
    for ch in range(2):
        a = 128 * ch
        b = 128 * (1 - ch)
        out[:, st_g(ch, 0):st_g(ch, 0) + 128] = G125[a:a + 128, a:a + 128]
        out[:, st_g(ch, 1):st_g(ch, 1) + 128] = G125[b:b + 128, a:a + 128]
    # lvl2/lvl3 butterfly stationaries
    for th in range(2):
        S = _perm_l2_th(0, th)
        O = _perm_l2_th(1, th)
        for neg in range(2):
            sgn = -1.0 if neg else 1.0
            out[:, st_l2(0, th, neg):st_l2(0, th, neg) + 128] = sgn * (S + O)
            out[:, st_l2(1, th, neg):st_l2(1, th, neg) + 128] = sgn * (S - O)
    S3 = _perm_l3(0)
    O3 = _perm_l3(1)
    for neg in range(2):
        sgn = -1.0 if neg else 1.0
        out[:, st_l3(0, neg):st_l3(0, neg) + 128] = sgn * (S3 + O3)
        out[:, st_l3(1, neg):st_l3(1, neg) + 128] = sgn * (S3 - O3)
    return out.astype(np.float16)


def _build():
    import concourse.bass as bass
    import concourse.mybir as mybir
    import concourse.tile as tile
    from contextlib import ExitStack

    dt = mybir.dt
    Alu = mybir.AluOpType
    Act = mybir.ActivationFunctionType
    F16, F32 = dt.float16, dt.float32

    T = THRESHOLD
    THR = (T / 4 * 2, T / 2 * 4, T * 8)

    nc = bass.Bass("TRN2", target_bir_lowering=False, debug=False,
                   num_devices=N_CORES)
    xsh = nc.dram_tensor("xs", [128, NPLANE, IMG, 258], F16, kind="ExternalInput")
    sth = nc.dram_tensor("st", [128, NSTAT * 128], F16, kind="ExternalInput")
    outh = nc.dram_tensor("res", [128, NACC], F32, kind="ExternalOutput")

    with tile.TileContext(nc) as tc, ExitStack() as ctx:
        v = nc.vector
        sc = nc.scalar

        pp = ctx.enter_context(tc.tile_pool(name="persist", bufs=1))
        xst = pp.tile([128, NPLANE, IMG, 258], F16, tag="xst")
        stats = pp.tile([128, NSTAT * 128], F16, tag="stats")
        out = pp.tile([128, 4, IMG, 512], F16, tag="out")     # [slot=t, m]
        Ee = pp.tile([128, 2, IMG, 512], F16, tag="Ee")       # [ph, m]
        Oo = pp.tile([128, 2, IMG, 512], F16, tag="Oo")
        sw = pp.tile([128, 2, IMG, 512], F16, tag="sw")       # [ph, m]
        dw = pp.tile([128, 2, IMG, 512], F16, tag="dw")
        ll1 = pp.tile([128, 2, IMG, 256], F16, tag="ll1")     # [th, m]
        det1 = pp.tile([128, 2, 3, 2, 2, 256], F16, tag="det1")  # [P, b, ph, mp]
        ll2 = pp.tile([128, IMG, 128], F16, tag="ll2")
        det2 = pp.tile([128, 3, IMG, 128], F16, tag="det2")
        det3 = pp.tile([128, 3, IMG, 64], F16, tag="det3")
        deadc = pp.tile([128, 2, 512], F16, tag="deadc")
        deadw = pp.tile([128, 2, 256], F16, tag="deadw")
        deadt = pp.tile([128, 3072], F16, tag="deadt")
        acc = pp.tile([128, NACC], F32, tag="acc")
        warm = pp.tile([128, 512], F16, tag="warm")

        ppre = ctx.enter_context(tc.tile_pool(name="ppre", bufs=4, space="PSUM"))
        pc = ctx.enter_context(tc.tile_pool(name="pc", bufs=2, space="PSUM"))

        # -------- input DMAs: one FIFO queue, consumer order --------
        nc.sync.dma_start(out=stats[:, 0:16 * 128], in_=sth.ap()[:, 0:16 * 128])
        nc.sync.dma_start(out=xst[:, 0:2, :, :], in_=xsh.ap()[:, 0:2])
        nc.sync.dma_start(out=xst[:, 2:4, :, :], in_=xsh.ap()[:, 2:4])
        nc.sync.dma_start(out=stats[:, 16 * 128:32 * 128],
                          in_=sth.ap()[:, 16 * 128:32 * 128])
        nc.sync.dma_start(out=xst[:, 4:8, :, :], in_=xsh.ap()[:, 4:8])
        nc.sync.dma_start(out=stats[:, 32 * 128:41 * 128],
                          in_=sth.ap()[:, 32 * 128:41 * 128])
        nc.sync.dma_start(out=xst[:, 8:12, :, :], in_=xsh.ap()[:, 8:12])
        nc.sync.dma_start(out=stats[:, 41 * 128:], in_=sth.ap()[:, 41 * 128:])

        v.memset(warm[:, :], 0.0)
        v.memset(acc[:, :], 0.0)

        # PE warm-up (p-state ramp) while inputs land
        wps = ppre.tile([128, 512], F32, name="wps", tag="pre")
        for _ in range(10):
            nc.tensor.matmul(wps[:, :], warm[:, 0:128], warm[:, :],
                             start=True, stop=True)

        # ---------------- conv units ----------------
        def conv_unit(m, half):
            pres = []
            for ti, t in enumerate(PAIRS[half]):
                pre = ppre.tile([128, 512], F32, name="pre", tag="pre")
                pres.append(pre)
                mv = xst[:, MOVP[t], m, :]
                for par in range(2):
                    # edge-correction 1-col matmul folded into the group:
                    # par0 fixes fine row 0 (free slot 0), par1 row 511 (511)
                    nc.tensor.matmul(
                        pre[:, 256 * par:256 * par + 256],
                        stats[:, st_conv(t, par, 0):st_conv(t, par, 0) + 128],
                        mv[:, 0:256], start=True, stop=False)
                    nc.tensor.matmul(
                        pre[:, 256 * par:256 * par + 256],
                        stats[:, st_conv(t, par, 1):st_conv(t, par, 1) + 128],
                        mv[:, 1:257], start=False, stop=False)
                    if par == 0:
                        nc.tensor.matmul(
                            pre[:, 0:1],
                            stats[:, st_corr(t, 0):st_corr(t, 0) + 128],
                            mv[:, 1:2], start=False, stop=False)
                    else:
                        nc.tensor.matmul(
                            pre[:, 511:512],
                            stats[:, st_corr(t, 1):st_corr(t, 1) + 128],
                            mv[:, 256:257], start=False, stop=False)
                    nc.tensor.matmul(
                        pre[:, 256 * par:256 * par + 256],
                        stats[:, st_conv(t, par, 2):st_conv(t, par, 2) + 128],
                        mv[:, 2:258], start=False, stop=True)
            if m in (1, 3):
                for ti, t in enumerate(PAIRS[half]):
                    v.tensor_scalar(out=out[:, t, m, :], in0=pres[ti][:, :],
                                    scalar1=0.0, scalar2=1.0,
                                    op0=Alu.max, op1=Alu.min)
            else:
                for ti, t in enumerate(PAIRS[half]):
                    sc.activation(out=out[:, t, m, :], in_=pres[ti][:, :],
                                  func=Act.Relu)

        def min_img(m):
            v.tensor_scalar(out=out[:, :, m, :], in0=out[:, :, m, :],
                            scalar1=1.0, scalar2=None, op0=Alu.min)

        # ---------------- c units ----------------
        def c_unit(m, half):
            ct = pc.tile([128, 2, 512], F32, name="ct", tag="ct")
            for ti, t in enumerate(PAIRS[half]):
                mv = xst[:, CPLANE[t], m, :]
                for par in range(2):
                    cols = slice(256 * par, 256 * par + 256)
                    for j, (dn, r) in enumerate(CTAPS[par]):
                        nc.tensor.matmul(ct[:, ti, cols],
                                         stats[:, st_c(t, par, j):st_c(t, par, j) + 128],
                                         mv[:, dn + 1:dn + 257],
                                         start=(j == 0), stop=False)
                    nc.tensor.matmul(ct[:, ti, cols],
                                     stats[:, ST_I:ST_I + 128],
                                     out[:, t, m, cols],
                                     start=False, stop=True)
            sc.activation(out=deadc[:, :, :], in_=ct[:, :, :], func=Act.Square,
                          accum_out=acc[:, 2 * m + half:2 * m + half + 1])

        # ---------------- delta term ----------------
        def delta_img(m):
            Wp = pc.tile([128, 2, 512], F32, name="wp", tag="ct")
            for ch in range(2):
                nc.tensor.matmul(Wp[:, ch, 0:256],
                                 stats[:, st_g(ch, 0):st_g(ch, 0) + 128],
                                 xst[:, 10 + ch, m, 1:257], start=True, stop=False)
                nc.tensor.matmul(Wp[:, ch, 0:256],
                                 stats[:, st_g(ch, 1):st_g(ch, 1) + 128],
                                 xst[:, 11 - ch, m, 1:257], start=False, stop=True)
            v.tensor_tensor(out=deadw[:, :, :], in0=xst[:, 8:10, m, 1:257],
                            in1=Wp[:, :, 0:256], op=Alu.mult)
            v.tensor_scalar(out=deadw[:, :, :], in0=deadw[:, :, :], scalar1=0.0,
                            scalar2=None, op0=Alu.bypass, op1=Alu.add,
                            accum_out=acc[:, 12 + m:13 + m])

        # ---------------- wavelet lvl1 ----------------
        def shuffle_pair(P):
            ms = slice(2 * P, 2 * P + 2)
            nc.sync.dma_start(out=Ee[0:64, :, ms, :],
                              in_=out[0:64, 0:4:2, ms, :])
            nc.sync.dma_start(out=Ee[64:128, :, ms, :],
                              in_=out[0:64, 1:4:2, ms, :])
            nc.sync.dma_start(out=Oo[0:64, :, ms, :],
                              in_=out[64:128, 0:4:2, ms, :])
            nc.sync.dma_start(out=Oo[64:128, :, ms, :],
                              in_=out[64:128, 1:4:2, ms, :])

        def lvl1_tt(P):
            ms = slice(2 * P, 2 * P + 2)
            eer = Ee[:, :, ms, :]
            oor = Oo[:, :, ms, :]
            v.tensor_tensor(out=sw[:, :, ms, :], in0=eer, in1=oor, op=Alu.add)
            v.tensor_tensor(out=dw[:, :, ms, :], in0=eer, in1=oor, op=Alu.subtract)
            v.tensor_tensor(out=ll1[:, :, ms, :], in0=sw[:, :, ms, 0:256],
                            in1=sw[:, :, ms, 256:512], op=Alu.add)
            v.tensor_tensor(out=det1[:, P, 0, :, :, :], in0=dw[:, :, ms, 0:256],
                            in1=dw[:, :, ms, 256:512], op=Alu.add)
            v.tensor_tensor(out=det1[:, P, 1, :, :, :], in0=sw[:, :, ms, 0:256],
                            in1=sw[:, :, ms, 256:512], op=Alu.subtract)
            v.tensor_tensor(out=det1[:, P, 2, :, :, :], in0=dw[:, :, ms, 0:256],
                            in1=dw[:, :, ms, 256:512], op=Alu.subtract)

        def lvl1_ts(P):
            flat = det1[:, P, :, :, :, :].rearrange("p b ph m f -> p (b ph m f)")
            sc.activation(out=flat, in_=flat, func=Act.Abs)
            v.tensor_scalar(out=deadt[:, 0:3072], in0=flat, scalar1=THR[0],
                            scalar2=None, op0=Alu.min, op1=Alu.add,
                            accum_out=acc[:, 8 + P:9 + P])

        def lvl1_ts4(P):
            # sum(min(|x|,t)) = sum(max(min(x,t),0)) - sum(min(max(x,-t),0))
            flat = det1[:, P, :, :, :, :].rearrange("p b ph m f -> p (b ph m f)")
            v.tensor_scalar(out=deadt[:, 0:3072], in0=flat, scalar1=THR[0],
                            scalar2=0.0, op0=Alu.min, op1=Alu.max)
            v.tensor_scalar(out=deadt[:, 0:3072], in0=deadt[:, 0:3072],
                            scalar1=0.0, scalar2=None, op0=Alu.bypass,
                            op1=Alu.add, accum_out=acc[:, 8 + P:9 + P])
            v.tensor_scalar(out=flat, in0=flat, scalar1=-THR[0],
                            scalar2=0.0, op0=Alu.max, op1=Alu.min)
            v.tensor_scalar(out=flat, in0=flat,
                            scalar1=0.0, scalar2=None, op0=Alu.bypass,
                            op1=Alu.add, accum_out=acc[:, 16 + P:17 + P])

        # ---------------- wavelet lvl2/lvl3 on PE ----------------
        def l1v(th, rp):
            return ll1[:, th, :, rp:256:2]

        def lvl2():
            pa0 = ppre.tile([128, 512], F32, name="pa0", tag="pre")
            pa1 = ppre.tile([128, 512], F32, name="pa1", tag="pre")
            pb0 = ppre.tile([128, 512], F32, name="pb0", tag="pre")
            pb1 = ppre.tile([128, 512], F32, name="pb1", tag="pre")
            bands = ((pa0, None, 0, (0, 0)), (pa1, None, 1, (0, 0)),
                     (pb0, None, 0, (0, 1)), (pb1, None, 1, (0, 1)))
            # (tile, slot, sd, rp-neg pattern): band0=ll2(sum,+,+),
            # band1=det2[0](dif,+,+), band2=det2[1](sum,+,-), band3=det2[2](dif,+,-)
            for bi, (tile_, slot, sd, negs) in enumerate(bands):
                base = tile_[:, :] if slot is None else tile_[:, slot, :]
                dstv = base.rearrange("p (m k) -> p m k", m=IMG)
                k = 0
                for rp in range(2):
                    for th in range(2):
                        nc.tensor.matmul(
                            dstv, stats[:, st_l2(sd, th, negs[rp]):
                                        st_l2(sd, th, negs[rp]) + 128],
                            l1v(th, rp), start=(k == 0), stop=(k == 3))
                        k += 1
            sc.activation(out=ll2[:, :, :],
                          in_=pa0[:, :].rearrange("p (m k) -> p m k", m=IMG),
                          func=Act.Copy)
            sc.activation(out=det2[:, 0, :, :],
                          in_=pa1[:, :].rearrange("p (m k) -> p m k", m=IMG),
                          func=Act.Abs)
            sc.activation(out=det2[:, 1, :, :],
                          in_=pb0[:, :].rearrange("p (m k) -> p m k", m=IMG),
                          func=Act.Abs)
            sc.activation(out=det2[:, 2, :, :],
                          in_=pb1[:, :].rearrange("p (m k) -> p m k", m=IMG),
                          func=Act.Abs)
            flat = det2[:, :, :, :].rearrange("p b m f -> p (b m f)")
            v.tensor_scalar(out=deadt[:, 0:1536], in0=flat, scalar1=THR[1],
                            scalar2=None, op0=Alu.min, op1=Alu.add,
                            accum_out=acc[:, 10:11])

        def l2v(rp):
            return ll2[:, :, rp:128:2]

        def lvl3():
            p3a = ppre.tile([128, 512], F32, name="p3a", tag="pre")
            p3b = ppre.tile([128, 512], F32, name="p3b", tag="pre")
            regions = (p3a[:, 0:256], p3a[:, 256:512], p3b[:, 0:256])
            for b in range(3):
                sd = (1, 0, 1)[b]
                negs = ((0, 0), (0, 1), (0, 1))[b]
                dstv = regions[b].rearrange("p (m k) -> p m k", m=IMG)
                for rp in range(2):
                    nc.tensor.matmul(
                        dstv, stats[:, st_l3(sd, negs[rp]):
                                    st_l3(sd, negs[rp]) + 128],
                        l2v(rp), start=(rp == 0), stop=(rp == 1))
                sc.activation(out=det3[:, b, :, :], in_=dstv, func=Act.Abs)
            flat = det3[:, :, :, :].rearrange("p b m f -> p (b m f)")
            v.tensor_scalar(out=deadt[:, 0:768], in0=flat, scalar1=THR[2],
                            scalar2=None, op0=Alu.min, op1=Alu.add,
                            accum_out=acc[:, 11:12])

        # ---------------- main schedule ----------------
        for P in range(2):
            m0, m1 = 2 * P, 2 * P + 1
            conv_unit(m0, 0)
            conv_unit(m1, 0)
            conv_unit(m0, 1)
            conv_unit(m1, 1)
            min_img(m0)
            shuffle_pair(P)
            lvl1_tt(P)
            if P == 1:
                lvl1_ts(0)
        for m in range(IMG):
            c_unit(m, 0)
            c_unit(m, 1)
            delta_img(m)
        lvl1_ts4(1)
        lvl2()
        lvl3()

        nc.sync.dma_start(out=outh.ap(), in_=acc[:, :])

    import os
    if not os.environ.get("SKIP_WAIT_SPLIT"):
        _split_multiwaits(nc, mybir)
    return nc


def _split_multiwaits(nc, mybir):
    """HW instructions support exactly ONE sync-wait; split extras into
    standalone Drains."""
    for f in nc.m.functions:
        for bb in f.blocks:
            i = 0
            while i < len(bb.instructions):
                ins = bb.instructions[i]
                si = getattr(ins, "sync_info", None)
                if si is not None and si.on_wait and len(si.on_wait) > 1:
                    waits = list(si.on_wait)
                    for w in waits[:-1]:
                        d = mybir.InstDrain(
                            name=nc.get_next_instruction_name(),
                            ins=[], outs=[], bass_is_fusable=False)
                        d.engine = ins.engine
                        d.sync_info = mybir.SyncInfo(on_wait=[w], on_update=[])
                        bb.instructions.insert(i, d)
                        i += 1
                    ins.sync_info = mybir.SyncInfo(
                        on_wait=[waits[-1]], on_update=list(si.on_update))
                i += 1


def _get_nc():
    if "nc" not in _CACHE:
        _CACHE["nc"] = _build()
    return _CACHE["nc"]


def make_in_maps(noisy_input, weight):
    x = np.asarray(noisy_input, dtype=np.float32).reshape(B_TOTAL, H, W)
    stats = _build_stats(weight)
    maps = []
    for c in range(N_CORES):
        xs = np.zeros((128, NPLANE, IMG, 258), dtype=np.float16)
        for m in range(IMG):
            img = x[c * IMG + m]
            p0 = img[0::2, 0::2].astype(np.float64)
            p3 = img[1::2, 1::2].astype(np.float64)
            qc = 2.0 * p0 + p3
            dl = p0 - p3
            g = np.concatenate([dl[:1], dl, dl[-1:]], axis=0)
            dp = g[1:257] + 0.3 * (g[0:256] + g[2:258])  # Gv/1.25 row factor
            for base, ph, wins in ((0, p0, True), (4, qc, True),
                                   (8, dl, False)):
                pt = np.ascontiguousarray(ph.T)          # [col, row]
                st = np.concatenate([pt[:, :1], pt, pt[:, -1:]], axis=1)
                xs[:, base + 0, m, :] = st[0:128]
                xs[:, base + 1, m, :] = st[128:256]
                if wins:
                    xs[:, base + 2, m, :] = st[62:190]   # Wa
                    xs[:, base + 3, m, :] = st[126:254]  # Wb
            dpt = np.ascontiguousarray(dp.T)
            xs[:, 10, m, 1:257] = dpt[0:128]
            xs[:, 11, m, 1:257] = dpt[128:256]
        maps.append({"xs": xs, "st": stats})
    return maps


def _host_combine(parts):
    S_c = S_d = 0.0
    wav = np.zeros(3)
    for p in parts:
        q = p.astype(np.float64)
        S_c += q[:, 0:8].sum()
        wav[0] += q[:, 8:10].sum() - q[:, 16:18].sum()
        wav[1] += q[:, 10].sum()
        wav[2] += q[:, 11].sum()
        S_d += q[:, 12:16].sum()
    N = B_TOTAL * H * W
    n2n = (3.0 * S_c + (2.0 / 3.0) * S_d) / N
    wtot = 0.0
    for j in (1, 2, 3):
        lvl = 3 - j + 1
        Nj = B_TOTAL * (H // 2 ** j) ** 2 * 3
        wtot += (1.0 / lvl) * (wav[j - 1] / (2.0 ** j)) / Nj
    return np.float32(n2n + WAVELET_WEIGHT * wtot)


def kernel(noisy_input, weight):
    from concourse.bass_utils import run_bass_kernel_spmd
    nc = _get_nc()
    in_maps = make_in_maps(noisy_input, weight)
    res = run_bass_kernel_spmd(nc, in_maps, list(range(N_CORES)))
    return _host_combine([r["res"] for r in res.results])
